# revision 1
# baseline (speedup 1.0000x reference)
"""CNF GNN message-passing kernel for trn2 (8 NeuronCores, SPMD).

Sharding: edges partitioned by DESTINATION (clause shard for l2c, literal
shard for c2l) so each core's local segment-sum is the full sum - no
all-reduce of node features. The linear-layer outputs (Wh tables) are
all-gathered so every core can gather rows for its edges.

Per iteration:
  stage A: Wh_l2c shard = tied_lembs @ Wl  (PE)          -> AllGather #1
  stage C: per 128-clause range: indirect-gather Wh_l2c rows per 128-edge
           chunk, build recip-scaled one-hot on DVE, accumulate
           psum[f, c] += gathered.T-style matmul; relu -> cembs_T
  stage D (fused): Wh_c2l = cembs @ Wc + clause_feat x wcf -> AllGather #2
  stage E: same aggregation into lembs_T per 128-literal range; pair-swap
           for literal tying; next stage A matmuls (or final transpose to
           the output on the last iteration).

Everything is f32; aggregation matmuls use exact fp32 (gathers are the
bottleneck at ~75 GB/s/core so PE/DVE have slack).
"""
import numpy as np
from dataclasses import dataclass

import concourse.bass as bass
import concourse.mybir as mybir
import concourse.tile as tile
from concourse.masks import make_identity

F32 = mybir.dt.float32
I32 = mybir.dt.int32
P = 128


# ---------------------------------------------------------------- host prep

@dataclass
class CoreData:
    in_map: dict          # name -> np.ndarray for this core
    n1: list              # chunks per clause slot (stage C)
    n2: list              # chunks per literal slot (stage E)
    perm1: np.ndarray = None   # slot -> real clause range
    perm2: np.ndarray = None   # slot -> real literal range


@dataclass
class Problem:
    L: int
    C: int
    E: int
    VLAB: int
    D: int                # VEMB == CEMB == 128
    ITERS: int
    ncores: int
    lsh_true: int         # literals per core (unpadded)
    csh_true: int
    LSH: int              # padded to multiple of 128
    CSH: int
    NR1: int              # clause ranges per core
    NR2: int              # literal ranges per core
    cores: list           # list[CoreData]


def _split_dir(dst_local, src_gidx, recip_edge, nranges, rng_width=P):
    """Sort edges by local destination; return per-range (gidx, rel, rc)
    unpadded arrays."""
    order = np.argsort(dst_local, kind="stable")
    dst_local = dst_local[order]
    src_gidx = src_gidx[order]
    recip_edge = recip_edge[order]
    bounds = np.searchsorted(dst_local, np.arange(nranges + 1) * rng_width)
    out = []
    for r in range(nranges):
        lo, hi = bounds[r], bounds[r + 1]
        out.append((
            src_gidx[lo:hi].astype(np.int32),
            (dst_local[lo:hi] - r * rng_width).astype(np.float32),
            recip_edge[lo:hi].astype(np.float32),
        ))
    return out


def _align_pack(per_core_ranges, nranges):
    """Align chunk counts across cores per range (max), pad, and pack into
    [P, NC] arrays per core. Returns (list of (idx, rel, rc) per core,
    common_nchunks)."""
    ncores = len(per_core_ranges)
    common = []
    for r in range(nranges):
        m = max(len(per_core_ranges[k][r][0]) for k in range(ncores))
        common.append((m + P - 1) // P)
    packed = []
    for k in range(ncores):
        idx_cols, rel_cols, rc_cols = [], [], []
        for r in range(nranges):
            nck = common[r]
            if nck == 0:
                continue
            gi, rl, rc = per_core_ranges[k][r]
            pad = nck * P - len(gi)
            gi = np.concatenate([gi, np.zeros(pad, np.int32)])
            rl = np.concatenate([rl, np.full(pad, -1.0, np.float32)])
            rcp = np.concatenate([rc, np.zeros(pad, np.float32)])
            idx_cols.append(gi.reshape(nck, P).T)
            rel_cols.append(rl.reshape(nck, P).T)
            rc_cols.append(rcp.reshape(nck, P).T)
        if idx_cols:
            packed.append((
                np.ascontiguousarray(np.concatenate(idx_cols, axis=1).astype(np.int32)),
                np.ascontiguousarray(np.concatenate(rel_cols, axis=1)),
                np.ascontiguousarray(np.concatenate(rc_cols, axis=1)),
            ))
        else:
            packed.append((np.zeros((P, 1), np.int32),
                           np.full((P, 1), -1.0, np.float32),
                           np.zeros((P, 1), np.float32)))
    return packed, common


def prepare(inputs, L, C, E, VLAB=8, D=128, ITERS=3, ncores=8, split_ag=True):
    lit_feat = np.asarray(inputs["lit_feat"], np.float32)
    clause_feat = np.asarray(inputs["clause_feat"], np.float32).reshape(-1)
    e_lit = np.asarray(inputs["edge_lit"], np.int32)
    e_cls = np.asarray(inputs["edge_clause"], np.int32)

    assert L % ncores == 0 and C % ncores == 0
    lsh_true, csh_true = L // ncores, C // ncores
    assert lsh_true % 2 == 0
    LSH = ((lsh_true + P - 1) // P) * P
    CSH = ((csh_true + P - 1) // P) * P
    NR1, NR2 = CSH // P, LSH // P

    # global gather index maps (account for per-core padding in AG layout).
    # With split_ag, the AG'd tables are HALF-MAJOR: [all cores' first-half
    # shard rows | all cores' second-half rows] so each half is the output
    # of its own (earlier-launchable) AllGather.
    LHALF, CHALF = LSH // 2, CSH // 2

    def glit(l):
        k, loc = l // lsh_true, l % lsh_true
        if not split_ag:
            return k * LSH + loc
        hi = loc >= LHALF
        return hi * (ncores * LHALF) + k * LHALF + (loc - hi * LHALF)

    def gcls(c):
        k, loc = c // csh_true, c % csh_true
        if not split_ag:
            return k * CSH + loc
        hi = loc >= CHALF
        return hi * (ncores * CHALF) + k * CHALF + (loc - hi * CHALF)

    cnt_c = np.bincount(e_cls, minlength=C).astype(np.float32)
    cnt_l = np.bincount(e_lit, minlength=L).astype(np.float32)
    rc_c = 1.0 / np.maximum(cnt_c, 1.0)
    rc_l = 1.0 / np.maximum(cnt_l, 1.0)

    W0 = np.asarray(inputs["W_l2c0"], np.float32)           # [VLAB, D]
    b0 = np.asarray(inputs["b_l2c0"], np.float32)           # [D]
    Wl = np.asarray(inputs["W_l2c"], np.float32)            # [ITERS-1, 2D, D]
    bl = np.asarray(inputs["b_l2c"], np.float32)            # [ITERS-1, D]
    Wc = np.asarray(inputs["W_c2l"], np.float32)            # [ITERS, D+1, D]
    bc = np.asarray(inputs["b_c2l"], np.float32)            # [ITERS, D]

    have_bias = bool(np.any(b0) or np.any(bl) or np.any(bc))
    # weights packed: Wc main [ITERS*D, D]; wcf rows [ITERS, D]; Wl halves
    Wc_main = np.concatenate([Wc[i, :D, :] for i in range(ITERS)], axis=0)
    wcf = np.stack([Wc[i, D, :] for i in range(ITERS)], axis=0)   # [ITERS, D]
    Wl_pack = Wl.reshape((ITERS - 1) * 2 * D, D) if ITERS > 1 else np.zeros((1, D), np.float32)
    bias_pack = np.concatenate([b0[None, :], bl if ITERS > 1 else np.zeros((0, D), np.float32), bc], axis=0)

    # pass 1: per-core destination-sorted ranges WITHOUT source indices yet
    raw1, raw2 = [], []
    for k in range(ncores):
        sel = (e_cls >= k * csh_true) & (e_cls < (k + 1) * csh_true)
        raw1.append((sel, _split_dir(
            (e_cls[sel] - k * csh_true).astype(np.int64),
            np.where(sel)[0].astype(np.int32),   # edge ids for now
            rc_c[e_cls[sel]], NR1)))
        sel2 = (e_lit >= k * lsh_true) & (e_lit < (k + 1) * lsh_true)
        raw2.append((sel2, _split_dir(
            (e_lit[sel2] - k * lsh_true).astype(np.int64),
            np.where(sel2)[0].astype(np.int32),
            rc_l[e_lit[sel2]], NR2)))

    # per-core slot permutation: sort ranges by descending edge count so the
    # cross-core per-slot max is tight
    perms1 = [np.argsort([-len(rr[0]) for rr in raw1[k][1]], kind="stable")
              for k in range(ncores)]
    perms2 = [np.argsort([-len(rr[0]) for rr in raw2[k][1]], kind="stable")
              for k in range(ncores)]
    inv1 = [np.argsort(p) for p in perms1]   # real range -> slot
    inv2 = [np.argsort(p) for p in perms2]

    # slot-aware global gather index maps (also half-major for split AG)
    NH1, NH2 = NR1 // 2, NR2 // 2

    def glit_slot(l):
        k = l // lsh_true
        loc = l % lsh_true
        r, w = loc // P, loc % P
        s = np.take(np.stack([inv2[j] for j in range(ncores)]), k * NR2 + r)
        hi = (s >= NH2).astype(np.int64)
        return (hi * (ncores * NH2 * P) + k * (NH2 * P)
                + (s - hi * NH2) * P + w)

    def gcls_slot(c):
        k = c // csh_true
        loc = c % csh_true
        r, w = loc // P, loc % P
        s = np.take(np.stack([inv1[j] for j in range(ncores)]), k * NR1 + r)
        hi = (s >= NH1).astype(np.int64)
        return (hi * (ncores * NH1 * P) + k * (NH1 * P)
                + (s - hi * NH1) * P + w)

    # pass 2: slot-ordered per-core range lists with REAL gather indices
    per_core_1, per_core_2 = [], []
    for k in range(ncores):
        lst = []
        for s in range(NR1):
            eids, rel, rc = raw1[k][1][perms1[k][s]]
            lst.append((glit_slot(e_lit[eids].astype(np.int64)).astype(np.int32),
                        rel, rc))
        per_core_1.append(lst)
        lst = []
        for s in range(NR2):
            eids, rel, rc = raw2[k][1][perms2[k][s]]
            lst.append((gcls_slot(e_cls[eids].astype(np.int64)).astype(np.int32),
                        rel, rc))
        per_core_2.append(lst)
    packed1, n1 = _align_pack(per_core_1, NR1)
    packed2, n2 = _align_pack(per_core_2, NR2)

    cores = []
    for k in range(ncores):
        idx1, rel1, rc1 = packed1[k]
        idx2, rel2, rc2 = packed2[k]
        litT0 = np.zeros((VLAB, LSH), np.float32)
        litT0[:, :lsh_true] = lit_feat[k * lsh_true:(k + 1) * lsh_true].T
        # slot-major columns: slot s holds real range perms2[k][s]
        litT = np.ascontiguousarray(
            litT0.reshape(VLAB, NR2, P)[:, perms2[k], :].reshape(VLAB, LSH))
        cf_shard = clause_feat[k * csh_true:(k + 1) * csh_true]
        cfk_flat = np.zeros(CSH, np.float32)
        cfk_flat[:csh_true] = cf_shard
        cfk = np.ascontiguousarray(
            cfk_flat.reshape(NR1, P)[perms1[k]].T)   # [P, NR1] slot-major

        in_map = dict(
            litT=litT, cf=cfk,
            idx1=idx1, rel1=rel1, rc1=rc1,
            idx2=idx2, rel2=rel2, rc2=rc2,
            W0=W0, Wc_main=Wc_main, wcf=wcf, Wl_pack=Wl_pack,
            bias_pack=bias_pack,
        )
        cores.append(CoreData(in_map, n1, n2, perms1[k], perms2[k]))

    p = Problem(L, C, E, VLAB, D, ITERS, ncores, lsh_true, csh_true,
                LSH, CSH, NR1, NR2, cores)
    p.split_ag = split_ag
    return p, have_bias


# ---------------------------------------------------------------- kernel

def build(prob: Problem, have_bias=False, gather_bufs=12, variant="full",
          oneh_bufs=6, work_bufs=4, psA_bufs=2, psD_bufs=2, psT_bufs=2,
          nqueues=1, single_packet=None):
    core_n1 = prob.cores[0].n1
    core_n2 = prob.cores[0].n2
    NC1 = prob.cores[0].in_map["idx1"].shape[1]
    NC2 = prob.cores[0].in_map["idx2"].shape[1]
    L, C, D, VLAB, ITERS = prob.L, prob.C, prob.D, prob.VLAB, prob.ITERS
    LSH, CSH, NR1, NR2 = prob.LSH, prob.CSH, prob.NR1, prob.NR2
    ncores = prob.ncores
    Lfull, Cfull = ncores * LSH, ncores * CSH
    lsh_true = prob.lsh_true

    nc = bass.Bass(num_swdge_queues=nqueues)
    # inputs
    litT = nc.dram_tensor("litT", [VLAB, LSH], F32, kind="ExternalInput")
    cf = nc.dram_tensor("cf", [P, NR1], F32, kind="ExternalInput")
    idx1 = nc.dram_tensor("idx1", [P, NC1], I32, kind="ExternalInput")
    rel1 = nc.dram_tensor("rel1", [P, NC1], F32, kind="ExternalInput")
    rc1 = nc.dram_tensor("rc1", [P, NC1], F32, kind="ExternalInput")
    idx2 = nc.dram_tensor("idx2", [P, NC2], I32, kind="ExternalInput")
    rel2 = nc.dram_tensor("rel2", [P, NC2], F32, kind="ExternalInput")
    rc2 = nc.dram_tensor("rc2", [P, NC2], F32, kind="ExternalInput")
    W0 = nc.dram_tensor("W0", [VLAB, D], F32, kind="ExternalInput")
    Wc_main = nc.dram_tensor("Wc_main", [ITERS * D, D], F32, kind="ExternalInput")
    wcf = nc.dram_tensor("wcf", [ITERS, D], F32, kind="ExternalInput")
    Wl_pack = nc.dram_tensor("Wl_pack", [max((ITERS - 1) * 2 * D, 1), D], F32, kind="ExternalInput")
    bias_pack = nc.dram_tensor("bias_pack", [2 * ITERS, D], F32, kind="ExternalInput")
    out = nc.dram_tensor("out", [LSH, 2 * D], F32, kind="ExternalOutput")

    # internal dram
    shared = "Shared" if ncores > 4 else "Local"
    whl_bounce_a = nc.dram_tensor("whl_bounce_a", [LSH // 2, D], F32)
    whl_bounce_b = nc.dram_tensor("whl_bounce_b", [LSH // 2, D], F32)
    whl_full = nc.dram_tensor("whl_full", [Lfull, D], F32, addr_space=shared)
    whc_bounce_a = nc.dram_tensor("whc_bounce_a", [CSH // 2, D], F32)
    whc_bounce_b = nc.dram_tensor("whc_bounce_b", [CSH // 2, D], F32)
    whc_full = nc.dram_tensor("whc_full", [Cfull, D], F32, addr_space=shared)

    rg = [list(range(ncores))]

    with tile.TileContext(nc) as tc:
        with (
            tc.tile_pool(name="const", bufs=1) as constp,
            tc.tile_pool(name="meta", bufs=1) as metap,
            tc.tile_pool(name="gath", bufs=gather_bufs) as gathp,
            tc.tile_pool(name="oneh", bufs=oneh_bufs) as onehp,
            tc.tile_pool(name="work", bufs=work_bufs) as workp,
            tc.tile_pool(name="outw", bufs=4) as outwp,
            tc.tile_pool(name="lit", bufs=3) as litp,
            tc.tile_pool(name="psA", bufs=psA_bufs, space="PSUM") as psA,
            tc.tile_pool(name="psD", bufs=psD_bufs, space="PSUM") as psD,
            tc.tile_pool(name="psT", bufs=psT_bufs, space="PSUM") as psT,
        ):
            # ---- constants
            iota_i = constp.tile([P, P], I32)
            nc.gpsimd.iota(iota_i[:], pattern=[[1, P]], base=0, channel_multiplier=0)
            iota_f = constp.tile([P, P], F32)
            nc.vector.tensor_copy(iota_f[:], iota_i[:])
            ident = constp.tile([P, P], F32)
            make_identity(nc, ident[:])

            W0_sb = constp.tile([VLAB, D], F32)
            nc.sync.dma_start(out=W0_sb[:], in_=W0[:, :])
            Wc_sb = constp.tile([P, ITERS * D], F32)
            for i in range(ITERS):
                nc.sync.dma_start(out=Wc_sb[:, i * D:(i + 1) * D],
                                  in_=Wc_main[i * P:(i + 1) * P, :])
            wcf_sb = constp.tile([1, ITERS * D], F32)
            for i in range(ITERS):
                nc.sync.dma_start(out=wcf_sb[:, i * D:(i + 1) * D], in_=wcf[i:i + 1, :])
            if ITERS > 1:
                Wl_sb = constp.tile([P, (ITERS - 1) * 2 * D], F32)
                for i in range(2 * (ITERS - 1)):
                    nc.sync.dma_start(out=Wl_sb[:, i * D:(i + 1) * D],
                                      in_=Wl_pack[i * P:(i + 1) * P, :])
            bias_sb = constp.tile([1, 2 * ITERS * D], F32)
            for i in range(2 * ITERS):
                nc.sync.dma_start(out=bias_sb[:, i * D:(i + 1) * D],
                                  in_=bias_pack[i:i + 1, :])
            ones_sb = constp.tile([1, P], F32)
            nc.vector.memset(ones_sb[:], 1.0)
            oh_const = constp.tile([P, P], F32)
            nc.vector.memset(oh_const[:], 0.01)
            cf_sb = constp.tile([P, NR1], F32)
            nc.sync.dma_start(out=cf_sb[:], in_=cf[:, :])

            # ---- edge metadata
            idx1_sb = metap.tile([P, NC1], I32)
            rel1_sb = metap.tile([P, NC1], F32)
            rc1_sb = metap.tile([P, NC1], F32)
            idx2_sb = metap.tile([P, NC2], I32)
            rel2_sb = metap.tile([P, NC2], F32)
            rc2_sb = metap.tile([P, NC2], F32)
            for dst, src in [(idx1_sb, idx1), (rel1_sb, rel1), (rc1_sb, rc1),
                             (idx2_sb, idx2), (rel2_sb, rel2), (rc2_sb, rc2)]:
                nc.sync.dma_start(out=dst[:], in_=src[:, :])

            def agg_direction(idx_sb, rel_sb, rc_sb, nchunks, nranges, table, c0s):
                """Yield (r, seg_T sbuf tile [P, P]) for each range after
                aggregation+relu. c0s[r] = starting chunk column."""
                for r in range(nranges):
                    nch = nchunks[r]
                    if nch == 0:
                        seg = workp.tile([P, P], F32, tag="seg")
                        nc.vector.memset(seg[:], 0.0)
                        yield r, seg
                        continue
                    ps = psA.tile([P, P], F32, space="PSUM", tag="agg")
                    for j in range(nch):
                        col = c0s[r] + j
                        g = gathp.tile([P, D], F32, tag="g")
                        if variant == "cheapdma":
                            nc.sync.dma_start(out=g[:], in_=table[0:P, :])
                        else:
                            gi = nc.gpsimd.indirect_dma_start(
                                out=g[:], out_offset=None, in_=table[:],
                                in_offset=bass.IndirectOffsetOnAxis(
                                    ap=idx_sb[:, col:col + 1], axis=0),
                            )
                            if single_packet is not None:
                                gi.ins.single_packet = single_packet
                            if nqueues > 1 and (col % nqueues):
                                gi.ins.queue = f"qPoolDynamic{col % nqueues}"

                        if variant == "nooh":
                            oh = oh_const
                        else:
                            oh = onehp.tile([P, P], F32, tag="oh")
                            nc.vector.tensor_scalar(
                                out=oh[:], in0=iota_f[:],
                                scalar1=rel_sb[:, col:col + 1],
                                scalar2=rc_sb[:, col:col + 1],
                                op0=mybir.AluOpType.is_equal,
                                op1=mybir.AluOpType.mult,
                            )
                        if variant != "nomm":
                            nc.tensor.matmul(out=ps[:], lhsT=g[:], rhs=oh[:],
                                             start=(j == 0), stop=(j == nch - 1))
                    if variant == "nomm":
                        seg = workp.tile([P, P], F32, tag="seg")
                        nc.vector.memset(seg[:], 0.01)
                    else:
                        seg = workp.tile([P, P], F32, tag="seg")
                        nc.vector.tensor_scalar_max(seg[:], ps[:], 0.0)
                    yield r, seg

            # chunk start columns
            c01 = np.cumsum([0] + core_n1).tolist()
            c02 = np.cumsum([0] + core_n2).tolist()

            split_ag = getattr(prob, "split_ag", False) and variant != "noag"
            LHALF, CHALF = LSH // 2, CSH // 2

            def emit_ag(bounces, full, sh, half):
                if variant == "noag":
                    return
                nc.gpsimd.collective_compute(
                    "AllGather", mybir.AluOpType.bypass,
                    ins=[bounces[half][:].opt()],
                    outs=[full[half * ncores * sh:(half + 1) * ncores * sh, :].opt()],
                    replica_groups=rg)

            def bounce_write(bounces, sh, r, tile_ap):
                half, off = (0, r * P) if r * P < sh else (1, r * P - sh)
                nc.sync.dma_start(out=bounces[half][off:off + P, :], in_=tile_ap)

            whl_bounces = (whl_bounce_a, whl_bounce_b)
            whc_bounces = (whc_bounce_a, whc_bounce_b)

            for it in range(ITERS):
                # ---------- stage A -> whl_bounce
                if it == 0:
                    for r in range(NR2):
                        lt = litp.tile([VLAB, P], F32, tag="lt")
                        nc.sync.dma_start(out=lt[:], in_=litT[:, r * P:(r + 1) * P])
                        ps = psD.tile([P, D], F32, space="PSUM", tag="whl")
                        nc.tensor.matmul(out=ps[:], lhsT=lt[:], rhs=W0_sb[:],
                                         start=True, stop=not have_bias)
                        if have_bias:
                            nc.tensor.matmul(out=ps[:], lhsT=ones_sb[:],
                                             rhs=bias_sb[:, 0:D],
                                             start=False, stop=True)
                        ot = outwp.tile([P, D], F32, tag="whl_o")
                        nc.scalar.activation(ot[:], ps[:],
                                             mybir.ActivationFunctionType.Copy)
                        bounce_write(whl_bounces, LHALF, r, ot[:])
                        if r == NR2 // 2 - 1:
                            emit_ag(whl_bounces, whl_full, LHALF, 0)
                    emit_ag(whl_bounces, whl_full, LHALF, 1)
                # (for it>0, stage A + its AGs were fused into stage E of it-1)
                if variant == "noag":
                    nc.sync.dma_start(out=whl_full[0:LSH // 2, :], in_=whl_bounce_a[:, :])
                    nc.sync.dma_start(out=whl_full[LSH // 2:LSH, :], in_=whl_bounce_b[:, :])

                # broadcast wcf[it] across partitions once per iteration
                ps_b = psD.tile([P, D], F32, space="PSUM", tag="whc")
                nc.tensor.matmul(out=ps_b[:], lhsT=ones_sb[:],
                                 rhs=wcf_sb[:, it * D:(it + 1) * D],
                                 start=True, stop=True)
                wcf_bc = workp.tile([P, D], F32, tag="wcfbc")
                nc.scalar.activation(wcf_bc[:], ps_b[:],
                                     mybir.ActivationFunctionType.Copy)

                # ---------- stage C + D: clause ranges
                for r, seg in agg_direction(idx1_sb, rel1_sb, rc1_sb,
                                            core_n1, NR1, whl_full, c01):
                    # Wh_c2l block: psum2[c, fo] = seg.T @ Wc[it]
                    ps2 = psD.tile([P, D], F32, space="PSUM", tag="whc")
                    nc.tensor.matmul(out=ps2[:], lhsT=seg[:],
                                     rhs=Wc_sb[:, it * D:(it + 1) * D],
                                     start=True, stop=not have_bias)
                    if have_bias:
                        nc.tensor.matmul(out=ps2[:], lhsT=ones_sb[:],
                                         rhs=bias_sb[:, (ITERS + it) * D:(ITERS + it + 1) * D],
                                         start=False, stop=True)
                    # + clause_feat[c] * wcf[it]
                    ot = outwp.tile([P, D], F32, tag="whc_o")
                    nc.vector.scalar_tensor_tensor(
                        out=ot[:], in0=wcf_bc[:], scalar=cf_sb[:, r:r + 1],
                        in1=ps2[:], op0=mybir.AluOpType.mult,
                        op1=mybir.AluOpType.add,
                    )
                    bounce_write(whc_bounces, CHALF, r, ot[:])
                    if r == NR1 // 2 - 1:
                        emit_ag(whc_bounces, whc_full, CHALF, 0)
                emit_ag(whc_bounces, whc_full, CHALF, 1)
                if variant == "noag":
                    nc.sync.dma_start(out=whc_full[0:CSH // 2, :], in_=whc_bounce_a[:, :])
                    nc.sync.dma_start(out=whc_full[CSH // 2:CSH, :], in_=whc_bounce_b[:, :])

                # ---------- stage E: literal ranges
                for r, seg in agg_direction(idx2_sb, rel2_sb, rc2_sb,
                                            core_n2, NR2, whc_full, c02):
                    # pair swap: partner literal embedding
                    swp = workp.tile([P, P], F32, tag="swp")
                    nc.vector.tensor_copy(swp[:, 0::2], seg[:, 1::2])
                    nc.vector.tensor_copy(swp[:, 1::2], seg[:, 0::2])
                    if it < ITERS - 1:
                        ps3 = psD.tile([P, D], F32, space="PSUM", tag="whl")
                        nc.tensor.matmul(out=ps3[:], lhsT=seg[:],
                                         rhs=Wl_sb[:, (2 * it) * D:(2 * it + 1) * D],
                                         start=True, stop=False)
                        nc.tensor.matmul(out=ps3[:], lhsT=swp[:],
                                         rhs=Wl_sb[:, (2 * it + 1) * D:(2 * it + 2) * D],
                                         start=False, stop=not have_bias)
                        if have_bias:
                            nc.tensor.matmul(out=ps3[:], lhsT=ones_sb[:],
                                             rhs=bias_sb[:, (1 + it) * D:(2 + it) * D],
                                             start=False, stop=True)
                        ot = outwp.tile([P, D], F32, tag="whl_o")
                        nc.scalar.activation(ot[:], ps3[:],
                                             mybir.ActivationFunctionType.Copy)
                        bounce_write(whl_bounces, LHALF, r, ot[:])
                        if r == NR2 // 2 - 1:
                            emit_ag(whl_bounces, whl_full, LHALF, 0)
                        if r == NR2 - 1:
                            emit_ag(whl_bounces, whl_full, LHALF, 1)
                    else:
                        # output: [own | partner] rows for this literal slot
                        pst = psT.tile([P, P], F32, space="PSUM", tag="tr")
                        nc.tensor.transpose(out=pst[:], in_=seg[:], identity=ident[:])
                        ob = outwp.tile([P, 2 * D], F32, tag="fin")
                        nc.scalar.activation(ob[:, :D], pst[:],
                                             mybir.ActivationFunctionType.Copy)
                        pst2 = psT.tile([P, P], F32, space="PSUM", tag="tr")
                        nc.tensor.transpose(out=pst2[:], in_=swp[:], identity=ident[:])
                        nc.scalar.activation(ob[:, D:], pst2[:],
                                             mybir.ActivationFunctionType.Copy)
                        nc.sync.dma_start(out=out[r * P:(r + 1) * P, :],
                                          in_=ob[:, :])
    return nc


def unpermute_out(prob: Problem, k, raw):
    """raw [LSH, 2D] slot-major -> [lsh_true, 2D] in real literal order."""
    NR2, perm = prob.NR2, prob.cores[k].perm2
    real = np.empty_like(raw)
    rr = raw.reshape(NR2, P, -1)
    real.reshape(NR2, P, -1)[perm] = rr
    return real[:prob.lsh_true]


def split_multiwait(nc, max_waits=1, verbose=False):
    import concourse.mybir as mb
    n_fix = 0
    for f in nc.m.functions:
        for b in f.blocks:
            new_insts = []
            for ins in b.instructions:
                si = getattr(ins, "sync_info", None)
                waits = list(si.on_wait) if (si and si.on_wait) else []
                if len(waits) > max_waits:
                    keep = waits[:max_waits]
                    extra = waits[max_waits:]
                    for i, w in enumerate(extra):
                        ev = mb.InstEventSemaphore(
                            name=f"{ins.name}-wsplit{i}",
                            engine=ins.engine,
                            ins=[],
                            outs=[],
                            sync_info=mb.SyncInfo(on_wait=[w], on_update=[]),
                        )
                        new_insts.append(ev)
                        try:
                            nc.register_instruction(ev)
                        except Exception:
                            nc.inst_map[ev.name] = ev
                    ins.sync_info = mb.SyncInfo(
                        on_wait=keep, on_update=list(si.on_update or [])
                    )
                    n_fix += 1
                new_insts.append(ins)
            b.instructions = new_insts
    if verbose:
        print(f"split_multiwait: fixed {n_fix} instructions")
    return nc


# ======================================================================
# harness entry point
# ======================================================================

def kernel(**inputs):
    """Full inputs in, full output out. Shards internally across 8 cores."""
    from concourse.bass_utils import run_bass_kernel_spmd

    NCORES = 8
    L, C, E = 100000, 200000, 800000
    prob, have_bias = prepare(inputs, L, C, E, VLAB=8, D=128, ITERS=3,
                              ncores=NCORES)
    nc = build(prob, have_bias=have_bias)
    split_multiwait(nc)
    res = run_bass_kernel_spmd(
        nc, [prob.cores[k].in_map for k in range(NCORES)],
        core_ids=list(range(NCORES)))
    out = np.concatenate(
        [unpermute_out(prob, k, res.results[k]["out"]) for k in range(NCORES)],
        axis=0).astype(np.float32)
    return out

